# revision 5
# baseline (speedup 1.0000x reference)
"""Trainium2 Bass kernel for a top-2 gated MoE layer (8 experts, H=1024, F=4096).

Strategy (expert parallelism, host routing/LN, balanced 2-slot sharding):
  - Host computes routing (top-2 of the fp32 gate logits), the softmax gate
    weights, and the per-token LayerNorm (incl. per-expert ln_w/ln_b apply).
  - The 8192 token-expert pairs are packed into 16 slots: 8 slots of width
    s1 and 8 of width s2 (one of each per core), each slot holding tokens of
    a single expert.  A small solver picks (s1, s2) so C = s1+s2 is minimal
    (big experts take two s1 slots, small ones two s2 slots, the rest one of
    each) -- C ends up ~1048 instead of max-expert-count padding (~1104).
  - Each core runs a pure fc1 -> gelu(tanh) -> fc2 pipeline over its two
    slots (each slot has its own expert weights), bf16 matmuls with fp32
    PSUM accumulation, streaming the weights in F-blocks of 512.
  - Host applies the gate weight + fc2 bias while scatter-adding the
    per-slot outputs back into the full [B,S,H] tensor.

Self-contained: shapes are hardcoded from the problem spec.
"""

import numpy as np
import ml_dtypes
from contextlib import ExitStack

TOP_K = 2
LN_EPS = 1e-5
B, S, H, E, F = 2, 2048, 1024, 8, 4096
T = B * S
P = 128
KH = H // P          # 8 H-tiles
FB = 512             # F block size (weight streaming granularity)
NFB = F // FB        # 8 blocks
MFB = FB // P        # 4 F-tiles per block

_BUILD_CACHE = {}


def _seg_chunks(s):
    """Split a segment of s columns into PSUM-sized (<=512) chunks.

    The first chunk is small (128) so the first fc1 group only waits on a
    tiny slice of the x DMA at kernel start."""
    if s <= 512:
        return [(0, s)]
    out = [(0, 128)]
    rem = s - 128
    n = (rem + 511) // 512
    off = 128
    for i in range(n):
        w = (rem + n - 1 - i) // n
        out.append((off, w))
        off += w
    return out


def _build(s1, s2):
    """Build + compile the single-core Bass program (SPMD across 8 cores)."""
    key = (s1, s2)
    if key in _BUILD_CACHE:
        return _BUILD_CACHE[key]

    import concourse.bass as bass  # noqa: F401
    import concourse.tile as tile
    import concourse.mybir as mybir
    from concourse import bacc

    bf = mybir.dt.bfloat16
    f32 = mybir.dt.float32
    AF = mybir.ActivationFunctionType

    C = s1 + s2
    segs = [(0, s1), (s1, s2)]  # (column offset, width) per slot

    nc = bacc.Bacc("TRN2", target_bir_lowering=False, debug=False,
                   num_devices=8)

    d_x = nc.dram_tensor("xnT", [P, KH, C], bf, kind="ExternalInput")
    d_w1 = [nc.dram_tensor(f"w1{s}", [NFB, P, KH, FB], bf,
                           kind="ExternalInput") for s in range(2)]
    d_w2 = [nc.dram_tensor(f"w2{s}", [NFB, P, MFB, H], bf,
                           kind="ExternalInput") for s in range(2)]
    d_b1 = nc.dram_tensor("b1r", [P, 2 * (F // P)], f32, kind="ExternalInput")
    d_y = nc.dram_tensor("ytT", [P, KH, C], f32, kind="ExternalOutput")

    with tile.TileContext(nc) as tc, ExitStack() as ctx:
        const = ctx.enter_context(tc.tile_pool(name="const", bufs=1))
        xpool = ctx.enter_context(tc.tile_pool(name="x", bufs=1))
        # bufs=2 on the weight pools: the ring's WAR hazard delays block
        # fb+1's DMA until block fb's tile has been consumed, so prefetch
        # traffic never competes with the startup-critical x/w1 transfers.
        w1pool = ctx.enter_context(tc.tile_pool(name="w1", bufs=2))
        w2pool = ctx.enter_context(tc.tile_pool(name="w2", bufs=2))
        apool = ctx.enter_context(tc.tile_pool(name="acts", bufs=2))
        ypool = ctx.enter_context(tc.tile_pool(name="yacc", bufs=1))
        ps1 = ctx.enter_context(tc.tile_pool(name="ps1", bufs=4, space="PSUM"))
        ps2 = ctx.enter_context(tc.tile_pool(name="ps2", bufs=4, space="PSUM"))

        # ---- warmup: ~3us of junk matmuls trains the PE clock gate to full
        # speed while the first x/w DMAs are in flight; also pre-load the
        # Gelu ACT table so the first real gelu doesn't stall on it. ----
        ones_k = const.tile([P, 1], bf)
        nc.vector.memset(ones_k, 1.0)
        warm_rhs = const.tile([P, 512], bf)
        nc.vector.memset(warm_rhs, 0.0)
        gtab = const.tile([P, 1], f32)
        nc.scalar.activation(gtab[:], ones_k[:], AF.Gelu_apprx_tanh)
        ps_w = ps1.tile([1, 512], f32, tag="ps1", name="warm")
        for i in range(9):
            nc.tensor.matmul(ps_w[:], ones_k[:], warm_rhs[:],
                             start=True, stop=True)

        # ---- input DMAs, most-critical first, each split across several
        # queues: x chunk0 + w1 slot0 block0 (per m-tile) gate the first
        # fc1 group; the rest follows. ----
        xsb = xpool.tile([P, KH, C], bf, tag="x", name="xsb")
        chunks0 = _seg_chunks(s1)
        ck0 = chunks0[0][1]
        for kh in range(0, KH, 4):
            nc.sync.dma_start(xsb[:, kh:kh + 4, 0:ck0],
                              d_x.ap()[:, kh:kh + 4, 0:ck0])

        def load_w(seg, fb, split_m=False):
            w1t = w1pool.tile([P, KH, FB], bf, tag="w1", name=f"w1_{seg}_{fb}")
            if split_m:
                for m in range(MFB):
                    nc.sync.dma_start(
                        w1t[:, :, m * P:(m + 1) * P],
                        d_w1[seg].ap()[fb][:, :, m * P:(m + 1) * P])
            else:
                nc.sync.dma_start(w1t[:], d_w1[seg].ap()[fb])
            w2t = w2pool.tile([P, MFB, H], bf, tag="w2", name=f"w2_{seg}_{fb}")
            nc.sync.dma_start(w2t[:], d_w2[seg].ap()[fb])
            return w1t, w2t

        w1t00 = w1pool.tile([P, KH, FB], bf, tag="w1", name="w1_0_0")
        for m in range(MFB):
            nc.sync.dma_start(w1t00[:, :, m * P:(m + 1) * P],
                              d_w1[0].ap()[0][:, :, m * P:(m + 1) * P])
        # rest of x, then slot0's w2, then slot1's weights
        for (off, w) in chunks0[1:]:
            nc.sync.dma_start(xsb[:, :, off:off + w],
                              d_x.ap()[:, :, off:off + w])
        if s2:
            nc.sync.dma_start(xsb[:, :, s1:C], d_x.ap()[:, :, s1:C])
        w2t00 = w2pool.tile([P, MFB, H], bf, tag="w2", name="w2_0_0")
        nc.sync.dma_start(w2t00[:], d_w2[0].ap()[0])
        wt = {(0, 0): (w1t00, w2t00)}
        if s2:
            wt[(1, 0)] = load_w(1, 0)

        b1sb = const.tile([P, 2 * (F // P)], f32)
        nc.sync.dma_start(b1sb[:], d_b1.ap())

        ysb = ypool.tile([P, KH, C], f32, tag="y", name="ysb")
        d_yr = d_y.ap()

        # ---- main pipeline: for each F-block, for each slot:
        # fc1 (all m-tiles) -> gelu -> fc2 (all h-tiles) -> y accumulate.
        # Last block runs slot1 first and its chunks reversed so the final
        # drain covers only the small 128-col chunk. ----
        for fb in range(NFB):
            # prefetch next block's weights (both slots; ring-throttled)
            if fb + 1 < NFB:
                wt[(0, fb + 1)] = load_w(0, fb + 1)
                if s2:
                    wt[(1, fb + 1)] = load_w(1, fb + 1)
            last = fb == NFB - 1
            seg_order = [1, 0] if last else [0, 1]
            for seg in seg_order:
                soff, swid = segs[seg]
                if not swid:
                    continue
                w1t, w2t = wt.pop((seg, fb))
                chunks = _seg_chunks(swid)
                if last:
                    chunks = chunks[::-1]
                asb = apool.tile([P, MFB, swid], bf, tag="acts",
                                 name=f"a_{seg}_{fb}")
                for (off, w) in chunks:
                    for m in range(MFB):
                        pst = ps1.tile([P, w], f32, tag="ps1",
                                       name=f"ps1_{fb}_{seg}_{m}_{off}")
                        for k in range(KH):
                            nc.tensor.matmul(
                                pst[:], w1t[:, k, m * P:(m + 1) * P],
                                xsb[:, k, soff + off:soff + off + w],
                                start=(k == 0), stop=(k == KH - 1))
                        fcol = seg * (F // P) + fb * MFB + m
                        nc.scalar.activation(asb[:, m, off:off + w], pst[:],
                                             AF.Gelu_apprx_tanh,
                                             bias=b1sb[:, fcol:fcol + 1])
                for (off, w) in chunks:
                    for h in range(KH):
                        pst = ps2.tile([P, w], f32, tag="ps2",
                                       name=f"ps2_{fb}_{seg}_{h}_{off}")
                        for m in range(MFB):
                            nc.tensor.matmul(
                                pst[:], w2t[:, m, h * P:(h + 1) * P],
                                asb[:, m, off:off + w],
                                start=(m == 0), stop=(m == MFB - 1))
                        ysl = ysb[:, h, soff + off:soff + off + w]
                        if fb == 0:
                            nc.scalar.activation(ysl, pst[:], AF.Identity,
                                                 bias=0.0)
                        else:
                            nc.vector.tensor_add(ysl, ysl, pst[:])
                            if last:
                                nc.sync.dma_start(
                                    d_yr[:, h:h + 1,
                                         soff + off:soff + off + w],
                                    ysb[:, h:h + 1,
                                        soff + off:soff + off + w])

    nc.compile()
    _BUILD_CACHE[key] = nc
    return nc


def _plan(counts):
    """Pick slot widths (s1, s2) and assign experts to the 16 slots.

    Config family indexed by x: the x biggest experts take two s1-slots,
    the x smallest take two s2-slots, the middle 8-2x take one of each.
    Returns (s1, s2, s1_pieces, s2_pieces) where each piece is
    (expert_id, n_tokens, token_offset_within_expert).
    """
    order = np.argsort(-np.asarray(counts), kind="stable")
    cs = [int(counts[e]) for e in order]

    best = None
    for x in range(0, 5):
        if x == 0:
            s1 = (cs[0] + 1) // 2
            s2 = cs[0] - s1
        else:
            s1 = (cs[0] + 1) // 2
            s2 = (cs[8 - x] + 1) // 2 if x >= 1 else 0
            mid = cs[x:8 - x]
            if mid and mid[0] > s1 + s2:
                s2 = mid[0] - s1
        s1 = max(s1, s2)
        cval = s1 + s2
        if best is None or cval < best[0]:
            best = (cval, x, s1, s2)
    _, x, s1, s2 = best
    # round up to multiples of 4 for DMA alignment
    s1 = (s1 + 3) // 4 * 4
    s2 = (s2 + 3) // 4 * 4

    s1_pieces, s2_pieces = [], []
    for i, e in enumerate(order):
        c = cs[i]
        if i < x:                       # two s1 slots
            a = (c + 1) // 2
            s1_pieces += [(int(e), a, 0), (int(e), c - a, a)]
        elif i >= 8 - x:                # two s2 slots
            a = (c + 1) // 2
            s2_pieces += [(int(e), a, 0), (int(e), c - a, a)]
        else:                           # one of each
            a = min(c, s1)
            s1_pieces.append((int(e), a, 0))
            s2_pieces.append((int(e), c - a, a))
    assert len(s1_pieces) == 8 and len(s2_pieces) == 8
    for (_, n, _o) in s1_pieces:
        assert n <= s1
    for (_, n, _o) in s2_pieces:
        assert n <= s2
    return s1, s2, s1_pieces, s2_pieces


def _prepare(x, Wg, alpha, ln_w, ln_b, fc1_w, fc1_b, fc2_w, fc2_b):
    """Host-side routing + LN + per-core input construction."""
    bfnp = ml_dtypes.bfloat16
    xf = np.asarray(x, np.float32).reshape(T, H)
    Wg = np.asarray(Wg, np.float32)
    alpha = np.asarray(alpha, np.float32)
    ln_w = np.asarray(ln_w, np.float32)
    ln_b = np.asarray(ln_b, np.float32)
    fc1_w = np.asarray(fc1_w, np.float32)
    fc1_b = np.asarray(fc1_b, np.float32)
    fc2_w = np.asarray(fc2_w, np.float32)
    fc2_b = np.asarray(fc2_b, np.float32)

    # routing (matches jax.lax.top_k tie-breaking: lowest index wins)
    logits = xf @ Wg
    order = np.argsort(-logits, axis=1, kind="stable")
    top2 = order[:, :TOP_K]
    tv = np.take_along_axis(logits, top2, axis=1)
    tv = tv - tv.max(axis=1, keepdims=True)
    ev = np.exp(tv)
    gsc = ev / ev.sum(axis=1, keepdims=True)          # [T, 2] softmax
    idx = [None] * E
    gw = [None] * E
    for e in range(E):
        sel = top2 == e                               # [T, 2]
        rows = np.nonzero(sel.any(axis=1))[0]
        idx[e] = rows
        gw[e] = gsc[rows][sel[rows]] * alpha[e]
    counts = [len(i) for i in idx]

    s1, s2, s1_pieces, s2_pieces = _plan(counts)
    C = s1 + s2

    # per-token LN (stats in fp32), per-expert scale/shift applied at gather
    mu = xf.mean(axis=1, keepdims=True)
    xc = xf - mu
    var = np.square(xc).mean(axis=1, keepdims=True)
    xn = xc / np.sqrt(var + LN_EPS)                   # [T, H]

    # per-expert packed weights (shared across cores via the same arrays)
    w1r = {}
    w2r = {}
    b1r = {}
    for e in set(p[0] for p in s1_pieces + s2_pieces):
        w1r[e] = np.ascontiguousarray(
            fc1_w[e].reshape(KH, P, NFB, FB).transpose(2, 1, 0, 3)
        ).astype(bfnp)
        w2r[e] = np.ascontiguousarray(
            fc2_w[e].reshape(NFB, FB // P, P, H).transpose(0, 2, 1, 3)
        ).astype(bfnp)
        b1r[e] = np.ascontiguousarray(fc1_b[e].reshape(F // P, P).T)

    in_maps = []
    meta = []
    for core in range(E):
        pieces = [s1_pieces[core], s2_pieces[core]]
        xg = np.zeros((C, H), np.float32)
        offs = [0, s1]
        for (slot, (e, n, toff)) in enumerate(pieces):
            if n:
                rows = idx[e][toff:toff + n]
                xg[offs[slot]:offs[slot] + n] = \
                    xn[rows] * ln_w[e] + ln_b[e]
        xnT = np.ascontiguousarray(
            xg.reshape(C, KH, P).transpose(2, 1, 0)).astype(bfnp)
        b1c = np.concatenate([b1r[pieces[0][0]], b1r[pieces[1][0]]], axis=1)
        in_maps.append({
            "xnT": xnT,
            "w10": w1r[pieces[0][0]], "w11": w1r[pieces[1][0]],
            "w20": w2r[pieces[0][0]], "w21": w2r[pieces[1][0]],
            "b1r": np.ascontiguousarray(b1c),
        })
        meta.append(pieces)
    return in_maps, meta, idx, gw, fc2_b, s1, s2


def _kernel_impl(inputs, trace=False, trace_cores=None):
    from concourse import bass_utils

    in_maps, meta, idx, gw, fc2_b, s1, s2 = _prepare(**inputs)
    nc = _build(s1, s2)
    res = bass_utils.run_bass_kernel_spmd(
        nc, in_maps, core_ids=list(range(E)),
        trace=trace, trace_cores=trace_cores)

    C = s1 + s2
    out = np.zeros((T, H), np.float32)
    offs = [0, s1]
    for core in range(E):
        yt = np.asarray(res.results[core]["ytT"], np.float32)  # [P, KH, C]
        yflat = yt.transpose(2, 1, 0).reshape(C, H)            # [C, H]
        for (slot, (e, n, toff)) in enumerate(meta[core]):
            if n:
                rows = idx[e][toff:toff + n]
                w = gw[e][toff:toff + n][:, None]
                out[rows] += w * (yflat[offs[slot]:offs[slot] + n]
                                  + fc2_b[e])
    return out.reshape(B, S, H), res


def kernel(**inputs):
    out, _ = _kernel_impl(inputs)
    return out


# revision 9
# speedup vs baseline: 1.2136x; 1.2136x over previous
"""Trainium2 Bass kernel for a top-2 gated MoE layer (8 experts, H=1024, F=4096).

Strategy (expert parallelism, host routing/LN, balanced 2-slot sharding):
  - Host computes routing (top-2 of the fp32 gate logits), the softmax gate
    weights, and the per-token LayerNorm (incl. per-expert ln_w/ln_b apply).
  - The 8192 token-expert pairs are packed into 16 slots: 8 slots of width
    s1 and 8 of width s2 (one of each per core), each slot holding tokens of
    a single expert.  A small solver picks (s1, s2) so C = s1+s2 is minimal
    (big experts take two s1 slots, small ones two s2 slots, the rest one of
    each) -- C ends up ~1048 instead of max-expert-count padding (~1104).
  - Each core runs a pure fc1 -> gelu(tanh) -> fc2 pipeline over its two
    slots (each slot has its own expert weights), bf16 matmuls with fp32
    PSUM accumulation, streaming the weights in F-blocks of 512.
  - Host applies the gate weight + fc2 bias while scatter-adding the
    per-slot outputs back into the full [B,S,H] tensor.

Self-contained: shapes are hardcoded from the problem spec.
"""

import numpy as np
import ml_dtypes
from contextlib import ExitStack

TOP_K = 2
LN_EPS = 1e-5
B, S, H, E, F = 2, 2048, 1024, 8, 4096
T = B * S
P = 128
KH = H // P          # 8 H-tiles
FB = 512             # F block size (weight streaming granularity)
NFB = F // FB        # 8 blocks
MFB = FB // P        # 4 F-tiles per block

_BUILD_CACHE = {}


def _seg_chunks(s):
    """Split a segment of s columns into PSUM-sized (<=512) chunks.

    The first chunk is small (128) so the first fc1 group only waits on a
    tiny slice of the x DMA at kernel start."""
    if s <= 512:
        return [(0, s)]
    out = [(0, 128)]
    rem = s - 128
    n = (rem + 511) // 512
    off = 128
    for i in range(n):
        w = (rem + n - 1 - i) // n
        out.append((off, w))
        off += w
    return out


def _build(s1, s2):
    """Build + compile the single-core Bass program (SPMD across 8 cores)."""
    key = (s1, s2)
    if key in _BUILD_CACHE:
        return _BUILD_CACHE[key]

    import concourse.bass as bass  # noqa: F401
    import concourse.tile as tile
    import concourse.mybir as mybir
    from concourse import bacc

    bf = mybir.dt.bfloat16
    f32 = mybir.dt.float32
    AF = mybir.ActivationFunctionType
    OP = mybir.AluOpType

    C = s1 + s2
    segs = [(0, s1), (s1, s2)]  # (column offset, width) per slot

    nc = bacc.Bacc("TRN2", target_bir_lowering=False, debug=False,
                   num_devices=8)

    d_x = nc.dram_tensor("xnT", [P, KH, C], bf, kind="ExternalInput")
    d_w1 = [nc.dram_tensor(f"w1{s}", [NFB, P, KH, FB], bf,
                           kind="ExternalInput") for s in range(2)]
    d_w2 = [nc.dram_tensor(f"w2{s}", [NFB, P, MFB, H], bf,
                           kind="ExternalInput") for s in range(2)]
    d_b1 = nc.dram_tensor("b1r", [P, 2 * (F // P)], f32, kind="ExternalInput")
    d_y = nc.dram_tensor("ytT", [P, KH, C], f32, kind="ExternalOutput")

    with tile.TileContext(nc) as tc, ExitStack() as ctx:
        const = ctx.enter_context(tc.tile_pool(name="const", bufs=1))
        xpool = ctx.enter_context(tc.tile_pool(name="x", bufs=1))
        # bufs=2 on the weight pools: the ring's WAR hazard delays block
        # fb+1's DMA until block fb's tile has been consumed, so prefetch
        # traffic never competes with the startup-critical x/w1 transfers.
        w1pool = ctx.enter_context(tc.tile_pool(name="w1", bufs=2))
        w2pool = ctx.enter_context(tc.tile_pool(name="w2", bufs=2))
        apool = ctx.enter_context(tc.tile_pool(name="acts", bufs=2))
        ypool = ctx.enter_context(tc.tile_pool(name="yacc", bufs=1))
        ps1 = ctx.enter_context(tc.tile_pool(name="ps1", bufs=4, space="PSUM"))
        ps2 = ctx.enter_context(tc.tile_pool(name="ps2", bufs=4, space="PSUM"))

        # ---- warmup: ~3us of junk matmuls trains the PE clock gate to full
        # speed while the first x/w DMAs are in flight; also pre-load the
        # Gelu ACT table so the first real gelu doesn't stall on it. ----
        ones_k = const.tile([P, 1], bf)
        nc.vector.memset(ones_k, 1.0)
        warm_rhs = const.tile([P, 512], bf)
        nc.vector.memset(warm_rhs, 0.0)
        gtab = const.tile([P, 1], f32)
        nc.scalar.activation(gtab[:], ones_k[:], AF.Gelu_apprx_tanh)
        ps_w = ps1.tile([1, 512], f32, tag="ps1", name="warm")
        for i in range(9):
            nc.tensor.matmul(ps_w[:], ones_k[:], warm_rhs[:],
                             start=True, stop=True)

        # ---- input DMAs, most-critical first: x chunk0 + b1 + w1 slot0
        # block0 (per m-tile) gate the first fc1 group.  The bulk transfers
        # (rest of x, w2, slot1 weights) are gated behind early fc1 gelu
        # outputs via tiny WAR-reader ops so their DMA traffic cannot starve
        # the startup-critical transfers of HBM bandwidth. ----
        xsb = xpool.tile([P, KH, C], bf, tag="x", name="xsb")
        chunks0 = _seg_chunks(s1)
        ck0 = chunks0[0][1]
        for kh in range(0, KH, 4):
            nc.sync.dma_start(xsb[:, kh:kh + 4, 0:ck0],
                              d_x.ap()[:, kh:kh + 4, 0:ck0])
        b1sb = const.tile([P, 2 * (F // P)], f32)
        nc.sync.dma_start(b1sb[:], d_b1.ap())

        def load_w(seg, fb, split_m=False):
            w1t = w1pool.tile([P, KH, FB], bf, tag="w1", name=f"w1_{seg}_{fb}")
            if split_m:
                for m in range(MFB):
                    nc.sync.dma_start(
                        w1t[:, :, m * P:(m + 1) * P],
                        d_w1[seg].ap()[fb][:, :, m * P:(m + 1) * P])
            else:
                nc.sync.dma_start(w1t[:], d_w1[seg].ap()[fb])
            w2t = w2pool.tile([P, MFB, H], bf, tag="w2", name=f"w2_{seg}_{fb}")
            nc.sync.dma_start(w2t[:], d_w2[seg].ap()[fb])
            return w1t, w2t

        w1t00 = w1pool.tile([P, KH, FB], bf, tag="w1", name="w1_0_0")
        for m in range(MFB):
            nc.sync.dma_start(w1t00[:, :, m * P:(m + 1) * P],
                              d_w1[0].ap()[0][:, :, m * P:(m + 1) * P])

        # deferred bulk transfers: issued in program order right after the
        # first fc1 gelu, behind a single gate (a tiny Vector op that reads
        # the first bulk-DMA's destination byte (WAR) and the first gelu's
        # output (RAW)).  The sync queue is in-order, so ALL later DMA
        # triggers queue behind the gate -- bulk traffic cannot starve the
        # startup-critical x-chunk0/w1/b1 transfers of HBM bandwidth.
        junkg = const.tile([1, 1], f32)
        gate_tok = xsb[0:1, 0, ck0:ck0 + 1]
        nc.vector.memset(gate_tok, 0.0)
        w2t00 = w2pool.tile([P, MFB, H], bf, tag="w2", name="w2_0_0")
        wt = {}
        wt[(0, 0)] = (w1t00, w2t00)

        def emit_bulk(asb0):
            # gate: WAR on x-chunk1's first byte + RAW on the first gelu
            nc.vector.tensor_tensor(junkg[:], gate_tok, asb0[0:1, 0, 0:1],
                                    OP.add)
            for (off, w) in chunks0[1:]:
                nc.sync.dma_start(xsb[:, :, off:off + w],
                                  d_x.ap()[:, :, off:off + w])
            nc.sync.dma_start(w2t00[:], d_w2[0].ap()[0])
            if s2:
                nc.sync.dma_start(xsb[:, :, s1:C], d_x.ap()[:, :, s1:C])
                wt[(1, 0)] = load_w(1, 0)

        ysb = ypool.tile([P, KH, C], f32, tag="y", name="ysb")
        d_yr = d_y.ap()

        # ---- main pipeline: for each F-block, for each slot:
        # fc1 (all m-tiles) -> gelu -> fc2 (all h-tiles) -> y accumulate.
        # Last block runs slot1 first and its chunks reversed so the final
        # drain covers only the small 128-col chunk. ----
        for fb in range(NFB):
            # prefetch next block's weights (both slots; ring-throttled).
            # For fb0 this is emitted after emit_bulk (inside the seg0 fc1
            # loop) -- putting it here would deadlock: its ring-WAR wait
            # (fc1-seg0-fb0 done) would block the in-order sync queue ahead
            # of the x-chunk1 trigger that fc1-seg0-fb0 itself needs.
            if 0 < fb < NFB - 1:
                wt[(0, fb + 1)] = load_w(0, fb + 1)
                if s2:
                    wt[(1, fb + 1)] = load_w(1, fb + 1)
            last = fb == NFB - 1
            seg_order = [1, 0] if last else [0, 1]
            for seg in seg_order:
                soff, swid = segs[seg]
                if not swid:
                    continue
                w1t, w2t = wt.pop((seg, fb))
                chunks = _seg_chunks(swid)
                if last:
                    chunks = chunks[::-1]
                asb = apool.tile([P, MFB, swid], bf, tag="acts",
                                 name=f"a_{seg}_{fb}")
                for (off, w) in chunks:
                    for m in range(MFB):
                        pst = ps1.tile([P, w], f32, tag="ps1",
                                       name=f"ps1_{fb}_{seg}_{m}_{off}")
                        for k in range(KH):
                            nc.tensor.matmul(
                                pst[:], w1t[:, k, m * P:(m + 1) * P],
                                xsb[:, k, soff + off:soff + off + w],
                                start=(k == 0), stop=(k == KH - 1))
                        fcol = seg * (F // P) + fb * MFB + m
                        nc.scalar.activation(asb[:, m, off:off + w], pst[:],
                                             AF.Gelu_apprx_tanh,
                                             bias=b1sb[:, fcol:fcol + 1])
                        if fb == 0 and seg == 0 and m == 0 and off == 0:
                            emit_bulk(asb)
                if fb == 0 and seg == 0:
                    # fb1 prefetch, safely behind the bulk gate
                    wt[(0, 1)] = load_w(0, 1)
                    if s2:
                        wt[(1, 1)] = load_w(1, 1)
                for (off, w) in chunks:
                    for h in range(KH):
                        pst = ps2.tile([P, w], f32, tag="ps2",
                                       name=f"ps2_{fb}_{seg}_{h}_{off}")
                        for m in range(MFB):
                            nc.tensor.matmul(
                                pst[:], w2t[:, m, h * P:(h + 1) * P],
                                asb[:, m, off:off + w],
                                start=(m == 0), stop=(m == MFB - 1))
                        ysl = ysb[:, h, soff + off:soff + off + w]
                        if fb == 0:
                            nc.scalar.activation(ysl, pst[:], AF.Identity,
                                                 bias=0.0)
                        else:
                            nc.vector.tensor_add(ysl, ysl, pst[:])
                            if last:
                                nc.sync.dma_start(
                                    d_yr[:, h:h + 1,
                                         soff + off:soff + off + w],
                                    ysb[:, h:h + 1,
                                        soff + off:soff + off + w])

    nc.compile()
    _BUILD_CACHE[key] = nc
    return nc


def _plan(counts):
    """Pick slot widths (s1, s2) and assign experts to the 16 slots.

    Config family indexed by x: the x biggest experts take two s1-slots,
    the x smallest take two s2-slots, the middle 8-2x take one of each.
    Returns (s1, s2, s1_pieces, s2_pieces) where each piece is
    (expert_id, n_tokens, token_offset_within_expert).
    """
    order = np.argsort(-np.asarray(counts), kind="stable")
    cs = [int(counts[e]) for e in order]

    best = None
    for x in range(0, 5):
        if x == 0:
            s1 = (cs[0] + 1) // 2
            s2 = cs[0] - s1
        else:
            s1 = (cs[0] + 1) // 2
            s2 = (cs[8 - x] + 1) // 2 if x >= 1 else 0
            mid = cs[x:8 - x]
            if mid and mid[0] > s1 + s2:
                s2 = mid[0] - s1
        s1 = max(s1, s2)
        cval = s1 + s2
        if best is None or cval < best[0]:
            best = (cval, x, s1, s2)
    _, x, s1, s2 = best
    # round up to multiples of 4 for DMA alignment
    s1 = (s1 + 3) // 4 * 4
    s2 = (s2 + 3) // 4 * 4

    s1_pieces, s2_pieces = [], []
    for i, e in enumerate(order):
        c = cs[i]
        if i < x:                       # two s1 slots
            a = (c + 1) // 2
            s1_pieces += [(int(e), a, 0), (int(e), c - a, a)]
        elif i >= 8 - x:                # two s2 slots
            a = (c + 1) // 2
            s2_pieces += [(int(e), a, 0), (int(e), c - a, a)]
        else:                           # one of each
            a = min(c, s1)
            s1_pieces.append((int(e), a, 0))
            s2_pieces.append((int(e), c - a, a))
    assert len(s1_pieces) == 8 and len(s2_pieces) == 8
    for (_, n, _o) in s1_pieces:
        assert n <= s1
    for (_, n, _o) in s2_pieces:
        assert n <= s2
    return s1, s2, s1_pieces, s2_pieces


def _prepare(x, Wg, alpha, ln_w, ln_b, fc1_w, fc1_b, fc2_w, fc2_b):
    """Host-side routing + LN + per-core input construction."""
    bfnp = ml_dtypes.bfloat16
    xf = np.asarray(x, np.float32).reshape(T, H)
    Wg = np.asarray(Wg, np.float32)
    alpha = np.asarray(alpha, np.float32)
    ln_w = np.asarray(ln_w, np.float32)
    ln_b = np.asarray(ln_b, np.float32)
    fc1_w = np.asarray(fc1_w, np.float32)
    fc1_b = np.asarray(fc1_b, np.float32)
    fc2_w = np.asarray(fc2_w, np.float32)
    fc2_b = np.asarray(fc2_b, np.float32)

    # routing (matches jax.lax.top_k tie-breaking: lowest index wins)
    logits = xf @ Wg
    order = np.argsort(-logits, axis=1, kind="stable")
    top2 = order[:, :TOP_K]
    tv = np.take_along_axis(logits, top2, axis=1)
    tv = tv - tv.max(axis=1, keepdims=True)
    ev = np.exp(tv)
    gsc = ev / ev.sum(axis=1, keepdims=True)          # [T, 2] softmax
    idx = [None] * E
    gw = [None] * E
    for e in range(E):
        sel = top2 == e                               # [T, 2]
        rows = np.nonzero(sel.any(axis=1))[0]
        idx[e] = rows
        gw[e] = gsc[rows][sel[rows]] * alpha[e]
    counts = [len(i) for i in idx]

    s1, s2, s1_pieces, s2_pieces = _plan(counts)
    C = s1 + s2

    # per-token LN (stats in fp32), per-expert scale/shift applied at gather
    mu = xf.mean(axis=1, keepdims=True)
    xc = xf - mu
    var = np.square(xc).mean(axis=1, keepdims=True)
    xn = xc / np.sqrt(var + LN_EPS)                   # [T, H]

    # per-expert packed weights (shared across cores via the same arrays)
    w1r = {}
    w2r = {}
    b1r = {}
    for e in set(p[0] for p in s1_pieces + s2_pieces):
        w1r[e] = np.ascontiguousarray(
            fc1_w[e].reshape(KH, P, NFB, FB).transpose(2, 1, 0, 3)
        ).astype(bfnp)
        w2r[e] = np.ascontiguousarray(
            fc2_w[e].reshape(NFB, FB // P, P, H).transpose(0, 2, 1, 3)
        ).astype(bfnp)
        b1r[e] = np.ascontiguousarray(fc1_b[e].reshape(F // P, P).T)

    in_maps = []
    meta = []
    for core in range(E):
        pieces = [s1_pieces[core], s2_pieces[core]]
        xg = np.zeros((C, H), np.float32)
        offs = [0, s1]
        for (slot, (e, n, toff)) in enumerate(pieces):
            if n:
                rows = idx[e][toff:toff + n]
                xg[offs[slot]:offs[slot] + n] = \
                    xn[rows] * ln_w[e] + ln_b[e]
        xnT = np.ascontiguousarray(
            xg.reshape(C, KH, P).transpose(2, 1, 0)).astype(bfnp)
        b1c = np.concatenate([b1r[pieces[0][0]], b1r[pieces[1][0]]], axis=1)
        in_maps.append({
            "xnT": xnT,
            "w10": w1r[pieces[0][0]], "w11": w1r[pieces[1][0]],
            "w20": w2r[pieces[0][0]], "w21": w2r[pieces[1][0]],
            "b1r": np.ascontiguousarray(b1c),
        })
        meta.append(pieces)
    return in_maps, meta, idx, gw, fc2_b, s1, s2


def _kernel_impl(inputs, trace=False, trace_cores=None):
    from concourse import bass_utils

    in_maps, meta, idx, gw, fc2_b, s1, s2 = _prepare(**inputs)
    nc = _build(s1, s2)
    res = bass_utils.run_bass_kernel_spmd(
        nc, in_maps, core_ids=list(range(E)),
        trace=trace, trace_cores=trace_cores)

    C = s1 + s2
    out = np.zeros((T, H), np.float32)
    offs = [0, s1]
    for core in range(E):
        yt = np.asarray(res.results[core]["ytT"], np.float32)  # [P, KH, C]
        yflat = yt.transpose(2, 1, 0).reshape(C, H)            # [C, H]
        for (slot, (e, n, toff)) in enumerate(meta[core]):
            if n:
                rows = idx[e][toff:toff + n]
                w = gw[e][toff:toff + n][:, None]
                out[rows] += w * (yflat[offs[slot]:offs[slot] + n]
                                  + fc2_b[e])
    return out.reshape(B, S, H), res


def kernel(**inputs):
    out, _ = _kernel_impl(inputs)
    return out
